# revision 38
# baseline (speedup 1.0000x reference)
"""Block Hadamard transform (128-wide blocks) on 8 Trainium2 NeuronCores.

y[..., n*128:(n+1)*128] = x[..., n*128:(n+1)*128] @ H  for the fixed
128x128 (already 1/sqrt(128)-scaled) Hadamard matrix H.

Strategy: uniform (rows, 128) @ (128, 128) matmul after viewing x as
block-rows of 128 contiguous elements.  Data-parallel shard of block-rows
across 8 cores; H replicated.  Per core the stream is DMA-bound
(32 MiB in + 32 MiB out at ~330 GB/s/core), so compute runs in fp16 to
keep the PE far under the DMA roofline (fp32 matmul is 4 cycles/row and
fp32 transpose 2, vs 1 cycle/row for fp16; the 2e-2 rel-err budget
dwarfs fp16's ~2.5e-4):

  per supertile of ch block-rows ("chunk" layout: partition p holds rows
  [p*ch, (p+1)*ch), one contiguous ch*512B DMA descriptor per partition):
    gpsimd (SWDGE) casting in-DMA fp32->fp16 straight into SBUF
    -> PE transpose (fp16, 1c/row) -> DVE copy PSUM->SBUF
    -> PE matmul vs fp16 H (1c/row, fp32 PSUM accumulate)
    -> DVE copy PSUM->SBUF fp32 -> out-DMA in 1MiB chunks alternating
       across the two HWDGE queues (SP + Activation).
  The PE program is software-pipelined (matmuls of group g-1 between
  transposes of groups g), and the casting in-DMA removes both the fp32
  SBUF staging and a separate cast op, so all engines stay hidden behind
  the HBM stream.

The DRAM output tensor is fp16 (host upcasts to fp32 after the gather):
with the 2e-2 rel-err budget (~6e-4 actual), halving the write traffic
moves the memory roofline itself — 3 MiB of HBM traffic per 4 MiB of
logical I/O (~213us -> ~168us measured).
"""

import contextlib

import numpy as np

import concourse.bass as bass  # noqa: F401  (registers engines)
import concourse.mybir as mybir
import concourse.tile as tile
from concourse import bacc
from concourse.bass_utils import run_bass_kernel_spmd
from concourse.masks import make_identity

N_CORES = 8
P = 128
FULL_SHAPE = (4, 4096, 4096)
S_TOTAL = int(np.prod(FULL_SHAPE)) // P  # 524288 block-rows
S = S_TOTAL // N_CORES                   # 65536 block-rows per core

F32 = mybir.dt.float32
CDTS = {
    "f16": mybir.dt.float16,
    "bf16": mybir.dt.bfloat16,
    "f32": mybir.dt.float32,
}

_CACHE: dict = {}


def _build(
    loop_repeat: int = 1,
    repeat: int = 1,
    ch: int = 32,            # block-row tiles per supertile (2 MiB/supertile)
    cdt: str = "f16",        # compute dtype for transpose+matmul
    layout: str = "chunk",   # chunk | interleave
    G: int = 8,              # transposes batched per PSUM bank / copy
    G2: int = 4,             # matmuls batched per PSUM bank / copy
    xbufs: int = 3,
    hbufs: int = 3,          # fp16 x buffers
    tbufs: int = 3,          # fp16 x^T sbuf buffers
    ybufs: int = 3,
    tpbufs: int = 2,
    ypbufs: int = 3,
    dma_mode: str = "split",  # split | dedicated | tri
    cast_splits: int = 2,
    compute: str = "full",    # full | none (DMA passthrough, timing only)
    in_cast: bool = False,    # gpsimd casting DMA on input (fp32->cdt)
    out_cast: bool = False,   # gpsimd casting DMA on output (cdt->fp32)
    out_splits: int = 2,      # out-DMA chunks per supertile
    in_splits: int = 1,       # in-DMA chunks per supertile (in_cast path)
    copy2: str = "vv",        # PSUM->SBUF y-copy engines (v=DVE a=Act p=Pool)
    ramp: int = 1,            # split first/last supertile into `ramp` pieces
    y16: bool = False,        # write y to DRAM as fp16 (host upcasts after
                              # gather; halves write traffic, err ~6e-4)
    s_rows: int | None = None,
    cores: int = N_CORES,
):
    cdt_t = CDTS[cdt]
    s = S if s_rows is None else s_rows
    nsuper = s // (P * ch)
    assert nsuper * P * ch == s
    nc = bacc.Bacc(
        "TRN2", target_bir_lowering=False, debug=False, num_devices=cores
    )
    ydt = CDTS["f16"] if y16 else F32
    xs = nc.dram_tensor("xs", [s, P], F32, kind="ExternalInput")
    hh = nc.dram_tensor("h", [P, P], F32, kind="ExternalInput")
    ys = nc.dram_tensor("ys", [s, P], ydt, kind="ExternalOutput")

    with tile.TileContext(nc) as tc:
        with (
            tc.tile_pool(name="consts", bufs=1) as consts,
            tc.tile_pool(name="xsup", bufs=xbufs) as xsup_pool,
            tc.tile_pool(name="xh", bufs=hbufs) as xh_pool,
            tc.tile_pool(name="tsb", bufs=tbufs) as tsb_pool,
            tc.tile_pool(name="ysup", bufs=ybufs) as ysup_pool,
            tc.tile_pool(name="tpsum", bufs=tpbufs, space="PSUM") as tpsum_pool,
            tc.tile_pool(name="ypsum", bufs=ypbufs, space="PSUM") as ypsum_pool,
        ):
            identity = consts.tile([P, P], cdt_t)
            make_identity(nc, identity[:])
            h_f32 = consts.tile([P, P], F32)
            nc.sync.dma_start(h_f32[:], hh[:, :])
            if cdt_t is F32:
                h_c = h_f32
            else:
                h_c = consts.tile([P, P], cdt_t)
                nc.gpsimd.tensor_copy(h_c[:], h_f32[:])

            # chunk: partition p holds block-rows [p*ch, (p+1)*ch) of the
            # supertile (one contiguous ch*512B descriptor per partition).
            # Block-rows are independent so any row->partition assignment
            # works as long as in/out use the same pattern.
            pattern = (
                "(j p) f -> p j f" if layout == "interleave" else "(p j) f -> p j f"
            )

            loop_cm = (
                tc.For_i(0, loop_repeat, 1)
                if loop_repeat > 1
                else contextlib.nullcontext()
            )
            with loop_cm:
                _body(
                    nc, xs, ys, xsup_pool, xh_pool, tsb_pool, ysup_pool,
                    tpsum_pool, ypsum_pool, identity, h_c, cdt_t,
                    nsuper, repeat, ch, G, G2, pattern, dma_mode, cast_splits,
                    compute, in_cast, out_cast, out_splits, copy2, in_splits,
                    ramp, y16,
                )

    nc.compile()
    return nc


def _copy(eng, dst, src):
    if hasattr(eng, "tensor_copy"):
        eng.tensor_copy(dst, src)
    else:
        eng.copy(dst, src)


def _body(
    nc, xs, ys, xsup_pool, xh_pool, tsb_pool, ysup_pool,
    tpsum_pool, ypsum_pool, identity, h_c, cdt_t,
    nsuper, repeat, ch, G, G2, pattern, dma_mode, cast_splits,
    compute="full", in_cast=False, out_cast=False, out_splits=2, copy2="vv",
    in_splits=1, ramp=1, y16=False,
):
    qs = (nc.sync, nc.scalar)       # the two HWDGE DMA queues
    engmap = {"v": nc.vector, "a": nc.scalar, "p": nc.gpsimd}
    copy2_engs = tuple(engmap[c] for c in copy2)

    # Schedule: list of (row_tile_start, width). Splitting the first/last
    # supertile into `ramp` pieces shortens pipeline fill/drain around the
    # per-iteration all-engine barrier.
    steps = []
    for i in range(nsuper):
        if ramp > 1 and i in (0, nsuper - 1):
            p = ch // ramp
            assert p % G == 0, "ramp pieces must be a multiple of G"
            steps.extend((i * ch + j * p, p) for j in range(ramp))
        else:
            steps.append((i * ch, ch))

    for rep_i in range(repeat):
        for si, (tstart, cw) in enumerate(steps):
            _super(
                nc, xs, ys, xsup_pool, xh_pool, tsb_pool, ysup_pool,
                tpsum_pool, ypsum_pool, identity, h_c, cdt_t,
                ch, cw, tstart, si, G, G2, pattern, dma_mode, cast_splits,
                compute, in_cast, out_cast, out_splits, copy2_engs,
                in_splits, qs, y16,
            )


def _super(
    nc, xs, ys, xsup_pool, xh_pool, tsb_pool, ysup_pool,
    tpsum_pool, ypsum_pool, identity, h_c, cdt_t,
    ch, cw, tstart, i, G, G2, pattern, dma_mode, cast_splits,
    compute, in_cast, out_cast, out_splits, copy2_engs, in_splits, qs,
    y16=False,
):
    rows = slice(tstart * P, (tstart + cw) * P)
    src = xs[rows, :].rearrange(pattern, p=P)
    if in_cast and cdt_t is not F32 and compute == "full":
        # gpsimd (SWDGE) DMA casts fp32->cdt in flight; no fp32 staging
        xh = xh_pool.tile([P, ch, P], cdt_t)
        isz = max(G, cw // in_splits)
        for c in range((cw + isz - 1) // isz):
            sl = slice(c * isz, min((c + 1) * isz, cw))
            nc.gpsimd.dma_start(xh[:, sl, :], src[:, sl, :])
        xt = None
    else:
        xt = xsup_pool.tile([P, ch, P], F32)
        if dma_mode == "split":
            half = cw // 2
            qs[i % 2].dma_start(xt[:, :half, :], src[:, :half, :])
            qs[(i + 1) % 2].dma_start(xt[:, half:cw, :], src[:, half:, :])
        elif dma_mode == "tri":
            # thirds over sync/scalar/gpsimd; rotate so each queue sees a
            # balanced mix of in+out halves across supertiles
            t3 = cw // 3 if (cw // 3) % 2 == 0 else cw // 3 + 1
            cuts = [0, t3, 2 * t3, cw]
            for j in range(3):
                sl = slice(cuts[j], cuts[j + 1])
                (nc.sync, nc.scalar, nc.gpsimd)[(i + j) % 3].dma_start(
                    xt[:, sl, :], src[:, sl, :]
                )
        else:
            qs[0].dma_start(xt[:, :cw, :], src)

    if compute == "none":
        # DMA passthrough: measures pure streaming rate (y = x, wrong
        # results on purpose — timing experiments only).
        yt = xt
    elif in_cast and cdt_t is not F32:
        pass
    elif cdt_t is F32:
        xh = xt
    else:
        xh = xh_pool.tile([P, ch, P], cdt_t)
        cs = cw // cast_splits
        for c in range(cast_splits):
            sl = slice(c * cs, (c + 1) * cs)
            nc.gpsimd.tensor_copy(xh[:, sl, :], xt[:, sl, :])

    dst = ys[rows, :].rearrange(pattern, p=P)

    if compute != "none":
        if y16:
            ydt = CDTS["f16"]
        elif out_cast and cdt_t is not F32:
            ydt = cdt_t
        else:
            ydt = F32
        yt = ysup_pool.tile([P, ch, P], ydt)
        ngroups = cw // G
        chunk = min(ch // out_splits, cw)
        emitted = 0

        def emit_out(done_tiles):
            nonlocal emitted
            while emitted + chunk <= done_tiles:
                sl = slice(emitted, emitted + chunk)
                cidx = emitted // chunk
                if out_cast and cdt_t is not F32:
                    oq = nc.gpsimd
                elif dma_mode == "dedicated" and not in_cast:
                    oq = qs[1]
                else:
                    oq = qs[(i + cidx) % 2]
                oq.dma_start(dst[:, sl, :], yt[:, sl, :])
                emitted += chunk

        def emit_matmuls(g, tsb):
            for h in range(G // G2):
                yp = ypsum_pool.tile([P, G2, P], F32)
                for k in range(G2):
                    nc.tensor.matmul(
                        yp[:, k, :], tsb[:, h * G2 + k, :], h_c[:],
                        start=True, stop=True,
                    )
                ysl = yt[:, g * G + h * G2 : g * G + (h + 1) * G2, :]
                _copy(copy2_engs[(g + h) % len(copy2_engs)], ysl, yp[:])

        pend = None
        for g in range(ngroups):
            tp = tpsum_pool.tile([P, G, P], cdt_t)
            for k in range(G):
                nc.tensor.transpose(
                    tp[:, k, :], xh[:, g * G + k, :], identity[:]
                )
            tsb = tsb_pool.tile([P, G, P], cdt_t)
            nc.vector.tensor_copy(tsb[:], tp[:])
            if pend is not None:
                emit_matmuls(pend[0], pend[1])
                emit_out((pend[0] + 1) * G)
            pend = (g, tsb)
        emit_matmuls(pend[0], pend[1])
        emit_out(cw)
    else:
        if dma_mode == "split":
            half = cw // 2
            qs[(i + 1) % 2].dma_start(dst[:, :half, :], yt[:, :half, :])
            qs[i % 2].dma_start(dst[:, half:, :], yt[:, half:cw, :])
        elif dma_mode == "tri":
            t3 = cw // 3 if (cw // 3) % 2 == 0 else cw // 3 + 1
            cuts = [0, t3, 2 * t3, cw]
            for j in range(3):
                sl = slice(cuts[j], cuts[j + 1])
                (nc.sync, nc.scalar, nc.gpsimd)[(i + j + 1) % 3].dma_start(
                    dst[:, sl, :], yt[:, sl, :]
                )
        else:
            qs[1].dma_start(dst, yt[:, :cw, :])


DEFAULT_CFG: dict = {
    "cdt": "f16",
    "in_cast": True,
    "ch": 64,
    "xbufs": 1,
    "hbufs": 3,
    "ybufs": 3,
    "tbufs": 2,
    "out_splits": 4,
    "in_splits": 2,
    "copy2": "vv",
    "y16": True,
}


def _get_nc():
    if "nc" not in _CACHE:
        _CACHE["nc"] = _build(**DEFAULT_CFG)
    return _CACHE["nc"]


def _run(x: np.ndarray, H: np.ndarray, trace: bool = False):
    nc = _get_nc()
    x_flat = np.ascontiguousarray(
        np.asarray(x, dtype=np.float32).reshape(S_TOTAL, P)
    )
    h_np = np.ascontiguousarray(np.asarray(H, dtype=np.float32))
    in_maps = [
        {"xs": x_flat[k * S : (k + 1) * S], "h": h_np} for k in range(N_CORES)
    ]
    try:
        res = run_bass_kernel_spmd(
            nc, in_maps, core_ids=list(range(N_CORES)), trace=trace
        )
    except ModuleNotFoundError:
        # This axon build has no NTFF profile hook (antenv.axon_hooks); if
        # tracing was requested via env (BASS_TRACE), fall back to untraced.
        import os

        os.environ["BASS_NEVER_TRACE"] = "1"
        res = run_bass_kernel_spmd(
            nc, in_maps, core_ids=list(range(N_CORES)), trace=False
        )
    y = np.concatenate([res.results[k]["ys"] for k in range(N_CORES)], axis=0)
    y = y.astype(np.float32, copy=False)  # upcast when device wrote fp16
    return y.reshape(FULL_SHAPE), res


def kernel(x: np.ndarray, H: np.ndarray) -> np.ndarray:
    y, _ = _run(x, H, trace=False)
    return y


if __name__ == "__main__":
    rng = np.random.default_rng(0)
    x = rng.standard_normal(FULL_SHAPE, dtype=np.float32)

    def _hadamard(n):
        h = np.array([[1.0]], dtype=np.float32)
        while h.shape[0] < n:
            h = np.block([[h, h], [h, -h]])
        return h

    H = (_hadamard(P) / np.sqrt(P)).astype(np.float32)
    y = kernel(x, H)
    expected = (x.reshape(-1, P) @ H).reshape(FULL_SHAPE)
    err = np.max(np.abs(y - expected)) / np.max(np.abs(expected))
    print("self-check rel err:", err)
